# revision 1
# baseline (speedup 1.0000x reference)
"""Adaptive per-pixel Gaussian smoothing (7x7, sigma from a sigmoid of a
perspective map) on 8 Trainium2 NeuronCores.

Strategy
--------
Shard: data-parallel over (batch, H-half): 4 batches x 2 halves = 8 cores.
Each core gets x_shard [64, 134, 262] (H halo of 3 + W zero-pad of 3 baked in
on the host) and persp_shard [128, 256], and produces out [64, 128, 256].

Math: per-pixel normalized 7x7 Gaussian weights factor as
    w[i,j](p) = e1(p)^(i^2+j^2) * invS2(p),   e1 = exp(-1/(2 sigma^2)),
    invS2 = (sum_i e1^(i^2))^-2  (the 2D tap-sum factorizes exactly).
i^2+j^2 takes only 10 values d in {0,1,2,4,5,8,9,10,13,18}, so
    out = sum_d u_d * C_d,   u_d = e1^d * invS2,
where C_d are *unweighted* ring sums of the shifted x. C_d are built from
symmetric column partial sums R_b = x<<b + x>>b (b=1,2,3 cols) with 21 adds
total instead of the naive 49 mul+48 add per-tap loop.

Engine split: ACT does the transcendental chain in pixel-major layout
(rows on partitions, 1/128 of the wall cost); TensorE broadcasts the ten
per-pixel weight maps across the 128 channel partitions with rank-1
(K=2 group-selector) fp32r matmuls into PSUM; DVE does only the ring adds
and the 10 mul + 9 add weighted combine per slab of 8 rows.

SBUF layout: partitions = 64 channels x 2 row-groups (rows 0-63 / 64-127 of
the shard), free dim = (row-in-slab, col) so all 7x7 shifts are free-dim AP
offsets and every DVE op runs on all 128 partitions.
"""

import numpy as np

import concourse.bass as bass
import concourse.tile as tile
from concourse import mybir
from concourse.bass_utils import run_bass_kernel_spmd

F32 = mybir.dt.float32
F32R = mybir.dt.float32r
AF = mybir.ActivationFunctionType
OP = mybir.AluOpType

B, C, H, W = 4, 64, 256, 256
NCORES = 8
HS = H // 2          # 128 rows per core
G = 2                # row groups per core (partitions = G*64 channels)
GR = HS // G         # 64 rows per group
S = 8                # slab rows
NSLAB = GR // S      # 8 slabs
WP = W + 6           # 262 padded cols
LN2 = 0.6931471805599453

DS = [0, 1, 2, 4, 5, 8, 9, 10, 13, 18]
RING = {0: [(0, 0)], 1: [(0, 1), (1, 0)], 2: [(1, 1)], 4: [(0, 4), (4, 0)],
        5: [(1, 4), (4, 1)], 8: [(4, 4)], 9: [(0, 9), (9, 0)],
        10: [(1, 9), (9, 1)], 13: [(4, 9), (9, 4)], 18: [(9, 9)]}

_CACHE = {}


def _build_nc():
    nc = bass.Bass()
    x_in = nc.declare_dram_parameter("x", [C, GR * G + 6, WP], F32, isOutput=False)
    p_in = nc.declare_dram_parameter("persp", [HS, W], F32, isOutput=False)
    abg_in = nc.declare_dram_parameter("abg", [128, 3], F32, isOutput=False)
    # row selectors for the PE broadcast: sels[k, i, m] = 1 iff the (slab,row)
    # index i pulls pixel-row k of u_d for out partition m (two group halves)
    sels_in = nc.declare_dram_parameter("sels", [128, GR, 128], F32, isOutput=False)
    id_in = nc.declare_dram_parameter("ident", [128, 128], F32, isOutput=False)
    out_d = nc.declare_dram_parameter("out", [C, HS, W], F32, isOutput=True)

    with tile.TileContext(nc) as tc:
        with (
            tc.tile_pool(name="const", bufs=1) as constp,
            tc.tile_pool(name="maps", bufs=1) as mapsp,
            tc.tile_pool(name="xw", bufs=2) as xwp,
            tc.tile_pool(name="rr", bufs=1) as rrp,
            tc.tile_pool(name="cd", bufs=1) as cdp,
            tc.tile_pool(name="tm", bufs=1) as tmp_,
            tc.tile_pool(name="ob", bufs=2) as obp,
            tc.tile_pool(name="ps", bufs=2, space="PSUM") as psp,
            tc.tile_pool(name="pso", bufs=1, space="PSUM") as psop,
        ):
            # ---------- preamble: constants ----------
            abg = constp.tile([128, 3], F32, tag="abg", name="abg")
            nc.gpsimd.dma_start(abg[:], abg_in[:])

            # row selectors for the broadcast matmuls: [K=128, i, M=128]
            sels = constp.tile([128, GR, 128], F32, tag="sels", name="sels")
            nc.gpsimd.dma_start(sels[:], sels_in[:])

            ident = constp.tile([128, 128], F32, tag="ident", name="ident")
            nc.gpsimd.dma_start(ident[:], id_in[:])

            nln2 = constp.tile([128, 1], F32, tag="nln2", name="nln2")
            nc.gpsimd.memset(nln2[:], -LN2)

            # ---------- preamble: per-pixel weight maps (pixel-major) ----------
            persp = mapsp.tile([128, W], F32, tag="persp", name="persp_sb")
            nc.gpsimd.dma_start(persp[:], p_in[:])

            def mtile(tag):
                return mapsp.tile([128, W], F32, tag=tag, name=tag)

            sg = mtile("sg")
            nc.scalar.activation(sg[:], persp[:], AF.Sigmoid,
                                 bias=abg[:, 2:3], scale=abg[:, 1:2])
            sig = mtile("sig")
            nc.vector.tensor_scalar(sig[:], sg[:], abg[:, 0:1], 1e-4,
                                    OP.mult, OP.max)
            lg = mtile("lg")
            nc.scalar.activation(lg[:], sig[:], AF.Ln)
            tt = mtile("tt")
            nc.scalar.activation(tt[:], lg[:], AF.Exp, bias=nln2[:], scale=-2.0)
            e = {}
            e[1] = mtile("e1")
            nc.scalar.activation(e[1][:], tt[:], AF.Exp, scale=-1.0)
            for d, (i, j) in ((2, (1, 1)), (4, (2, 2)), (5, (4, 1)), (8, (4, 4)),
                              (9, (8, 1)), (10, (8, 2)), (13, (9, 4)), (18, (9, 9))):
                e[d] = mtile(f"e{d}")
                nc.vector.tensor_mul(e[d][:], e[i][:], e[j][:])
            ssum = mtile("ssum")
            nc.vector.tensor_add(ssum[:], e[1][:], e[4][:])
            nc.vector.tensor_add(ssum[:], ssum[:], e[9][:])
            sv = mtile("sv")
            nc.vector.tensor_scalar(sv[:], ssum[:], 2.0, 1.0, OP.mult, OP.add)
            l2 = mtile("l2")
            nc.scalar.activation(l2[:], sv[:], AF.Ln)
            u = {}
            u[0] = mtile("u0")
            nc.scalar.activation(u[0][:], l2[:], AF.Exp, scale=-2.0)
            for d in DS[1:]:
                u[d] = mtile(f"u{d}")
                nc.vector.tensor_mul(u[d][:], e[d][:], u[0][:])

            # ---------- main loop over slabs ----------
            for s in range(NSLAB):
                xw = xwp.tile([128, S + 6, WP], F32, tag="xw", name="xw")
                for g in range(G):
                    nc.gpsimd.dma_start(
                        xw[64 * g:64 * (g + 1), :, :],
                        x_in[:, g * GR + s * S: g * GR + s * S + S + 6, :])

                # symmetric column partial sums (valid cols 3..3+W)
                R = {0: xw}
                for b, r in ((1, 1), (4, 2), (9, 3)):
                    Rb = rrp.tile([128, S + 6, WP], F32, tag=f"R{b}", name=f"R{b}")
                    nc.vector.tensor_add(Rb[:, :, 3:3 + W],
                                         xw[:, :, 3 - r:3 - r + W],
                                         xw[:, :, 3 + r:3 + r + W])
                    R[b] = Rb

                H2 = S // 2
                oacc = [psop.tile([128, H2, W], F32, tag=f"oacc{h}",
                                  name=f"oacc{h}") for h in range(2)]

                pend_acc = []

                def flush_acc(last=False):
                    for tm, h, first in pend_acc:
                        for q in range(2):
                            nc.tensor.matmul(
                                oacc[h][:, q * (H2 // 2):(q + 1) * (H2 // 2), :],
                                ident[:],
                                tm[:, q * (H2 // 2):(q + 1) * (H2 // 2), :],
                                start=first, stop=last,
                                skip_group_check=True)
                    pend_acc.clear()

                for di, d in enumerate(DS):
                    # broadcast u_d rows across the 128 (group, channel)
                    # partitions via selector matmuls into half-slab PSUM
                    # tiles (bufs=2 so PE pipelines ahead of the DVE muls)
                    ureps = []
                    for h in range(2):
                        ur = psp.tile([128, H2, W], F32, tag="urep",
                                      name="urep")
                        ureps.append(ur)
                        for r2 in range(H2):
                            row = s * S + h * H2 + r2
                            nc.tensor.matmul(
                                ur[:, r2, :],
                                sels[:, row, :],
                                u[d][:],
                                start=True, stop=True)
                    flush_acc()

                    # ring sum C_d
                    if d == 0:
                        cd_ap = xw[:, 3:3 + S, 3:3 + W]
                    else:
                        cd = cdp.tile([128, S, W], F32, tag="cd", name="cd")
                        first = True
                        pend = None
                        for (a, b) in RING[d]:
                            ra = int(np.sqrt(a))
                            if a == 0:
                                pend = R[b][:, 3:3 + S, 3:3 + W]
                                continue
                            if first:
                                nc.vector.tensor_add(
                                    cd[:],
                                    R[b][:, 3 - ra:3 - ra + S, 3:3 + W],
                                    R[b][:, 3 + ra:3 + ra + S, 3:3 + W])
                                first = False
                            else:
                                tb = cdp.tile([128, S, W], F32, tag="tb", name="tb")
                                nc.vector.tensor_add(
                                    tb[:],
                                    R[b][:, 3 - ra:3 - ra + S, 3:3 + W],
                                    R[b][:, 3 + ra:3 + ra + S, 3:3 + W])
                                nc.vector.tensor_add(cd[:], cd[:], tb[:])
                        if pend is not None:
                            nc.vector.tensor_add(cd[:], cd[:], pend)
                        cd_ap = cd[:]

                    # weighted product on DVE; sum over d accumulated by
                    # PE identity matmuls into PSUM. Accumulation for ring d
                    # is emitted AFTER the next ring's broadcasts (pend_acc)
                    # so the in-order PE queue never head-of-line blocks on
                    # the DVE product it consumes.
                    for h in range(2):
                        tm = tmp_.tile([128, H2, W], F32, tag=f"tm{h}",
                                       name=f"tm{h}", bufs=2)
                        if d == 0:
                            ca = xw[:, 3 + h * H2:3 + h * H2 + H2, 3:3 + W]
                        else:
                            ca = cd_ap[:, h * H2:(h + 1) * H2, :]
                        us = tmp_.tile([128, H2, W], F32, tag=f"us{h}",
                                       name=f"us{h}", bufs=2)
                        nc.scalar.copy(us[:], ureps[h][:])
                        nc.vector.tensor_mul(tm[:], ca, us[:])
                        pend_acc.append((tm, h, di == 0))

                flush_acc(last=True)
                out_sb = obp.tile([128, S, W], F32, tag="ob", name="ob")
                for h in range(2):
                    nc.scalar.copy(out_sb[:, h * H2:(h + 1) * H2, :],
                                   oacc[h][:])
                for g in range(G):
                    nc.gpsimd.dma_start(
                        out_d[:, g * GR + s * S: g * GR + s * S + S, :],
                        out_sb[64 * g:64 * (g + 1), :, :])
    return nc


def _selectors():
    """sels[k, i, m] = 1 iff pixel-row k feeds out partition m at row index i."""
    if "sels" not in _CACHE:
        sels = np.zeros((128, GR, 128), np.float32)
        for i in range(GR):
            sels[i, i, 0:64] = 1.0          # group 0: pixel row i
            sels[GR + i, i, 64:128] = 1.0   # group 1: pixel row 64+i
        _CACHE["sels"] = sels
    return _CACHE["sels"]


def _split_waits(nc):
    """Walrus on this toolchain accepts only one semaphore wait per compute
    instruction; hoist excess waits onto same-engine NoOps placed before."""
    for f in nc.m.functions:
        for bb in f.blocks:
            new_list = []
            for ins in bb.instructions:
                si = ins.sync_info
                if si is not None and len(si.on_wait) > 1:
                    waits = list(si.on_wait)
                    for k, w in enumerate(waits[:-1]):
                        nop = mybir.InstNoOp(name=f"{ins.name}-ws{k}",
                                             ins=[], outs=[])
                        nop.engine = ins.engine
                        nop.sync_info = mybir.SyncInfo(on_wait=[w], on_update=[])
                        new_list.append(nop)
                    ins.sync_info = mybir.SyncInfo(on_wait=[waits[-1]],
                                                  on_update=list(si.on_update))
                new_list.append(ins)
            bb.instructions = new_list


def _get_nc():
    if "nc" not in _CACHE:
        nc = _build_nc()
        _split_waits(nc)
        _CACHE["nc"] = nc
    return _CACHE["nc"]


def kernel(x, perspective, alpha, beta, gamma, kernel_size):
    assert int(kernel_size) == 7
    x = np.ascontiguousarray(np.asarray(x, dtype=np.float32))
    perspective = np.asarray(perspective, dtype=np.float32)
    a = np.float32(np.asarray(alpha).reshape(-1)[0])
    bt = np.float32(np.asarray(beta).reshape(-1)[0])
    gm = np.float32(np.asarray(gamma).reshape(-1)[0])
    abg = np.broadcast_to(np.array([a, bt, gm], np.float32), (128, 3)).copy()
    sels = _selectors()

    xp = np.pad(x, ((0, 0), (0, 0), (3, 3), (3, 3)))
    in_maps = []
    for b in range(B):
        for half in range(2):
            r0 = half * HS
            in_maps.append({
                "x": np.ascontiguousarray(xp[b, :, r0:r0 + HS + 6, :]),
                "persp": np.ascontiguousarray(perspective[b, 0, r0:r0 + HS, :]),
                "abg": abg,
                "sels": sels,
                "ident": np.eye(128, dtype=np.float32),
            })

    nc = _get_nc()
    res = run_bass_kernel_spmd(nc, in_maps, list(range(NCORES)))
    _CACHE["last_res"] = res
    out = np.empty((B, C, H, W), np.float32)
    k = 0
    for b in range(B):
        for half in range(2):
            out[b, :, half * HS:(half + 1) * HS, :] = res.results[k]["out"]
            k += 1
    return out


if __name__ == "__main__":
    rng = np.random.default_rng(0)
    x = rng.standard_normal((B, C, H, W), dtype=np.float32)
    persp = rng.random((B, 1, H, W), dtype=np.float32)
    o = kernel(x=x, perspective=persp, alpha=np.ones(1, np.float32) * 3,
               beta=np.ones(1, np.float32), gamma=np.zeros(1, np.float32),
               kernel_size=7)
    print(o.shape, o.dtype, float(np.abs(o).mean()))

